# revision 1
# baseline (speedup 1.0000x reference)
"""Trainium2 Bass kernel for GraphTransformerNet (star-graph TransformerConv).

Shapes (hardcoded): B=1024 graphs, N=128 neighbors, D=256 in-dim,
H=4 heads x C=64 = F=256 out-dim. Data-parallel over 8 NeuronCores
(128 graphs/core). Host pre-transposes inputs to [B, D, N] bf16 so the
contraction dim lands on SBUF partitions with zero on-chip transposes.
The tiny q projection (0.05% of FLOPs) is done host-side and shipped as
pre-masked "Qblock" columns; the central skip projection seeds the
aggregation PSUM accumulator so no final add is needed.

Per graph g (per core):
  kT[f,n] = Wk.T @ xT_g + We.T @ eT_g          (PSUM-accumulated, batched x4 graphs)
  v[n,f]  = xT_g.T @ Wv + eT_g.T @ We          } one [128,512] psum: [v | skip_n]
  skip[n,f] = xT_g.T @ Wskip                   }
  scoresT[n,h] = kT_chunk.T @ Qblock_chunk     (2 matmuls, 2 cols each)
  softmax over n: packed 8 graphs -> [32,128] rows=(g,h), PE-transposed
  agg_ps[f_loc,fc,g] (+)= v_slice.T @ alphaT_col  (4 tiny matmuls/graph,
        accumulated on top of Wskip.T @ central seeded at start)
  central row = agg_ps transposed back at the end
"""

import sys

import numpy as np

for _p in ("/opt/trn_rl_repo",):
    if _p not in sys.path:
        sys.path.insert(0, _p)

import ml_dtypes

import concourse.bacc as bacc
import concourse.bass as bass
import concourse.mybir as mybir
from concourse.bass import MemorySpace
from concourse.tile import TileContext

BF16 = mybir.dt.bfloat16
F32 = mybir.dt.float32
AFT = mybir.ActivationFunctionType

B, N, D, H, C = 1024, 128, 256, 4, 64
F = H * C            # 256
NCORES = 8
BG = B // NCORES     # 128 graphs per core
GROUP = 8            # graphs per softmax pack
NB4 = 4              # graphs per kT matmul batch
ROWS = N + 1         # 129 output rows per graph

_cached = {}


def _build_nc():
    nc = bacc.Bacc()

    xt_d = nc.dram_tensor("xt", [BG, D, N], BF16, kind="ExternalInput")
    et_d = nc.dram_tensor("et", [BG, D, N], BF16, kind="ExternalInput")
    ct_d = nc.dram_tensor("ct", [D, BG], BF16, kind="ExternalInput")
    qb0_d = nc.dram_tensor("qb0", [128, BG, 2], BF16, kind="ExternalInput")
    qb1_d = nc.dram_tensor("qb1", [128, BG, 2], BF16, kind="ExternalInput")
    wk_d = nc.dram_tensor("wk", [D, F], BF16, kind="ExternalInput")
    we_d = nc.dram_tensor("we", [D, F], BF16, kind="ExternalInput")
    wvs_d = nc.dram_tensor("wvs", [D, 2 * F], BF16, kind="ExternalInput")
    idb_d = nc.dram_tensor("idb", [128, 128], BF16, kind="ExternalInput")
    idf_d = nc.dram_tensor("idf", [128, 128], F32, kind="ExternalInput")
    out_d = nc.dram_tensor("out", [BG * ROWS, F], F32, kind="ExternalOutput")

    out_rows = out_d[:, :].rearrange("(g r) f -> g r f", r=ROWS)

    with TileContext(nc) as tc:
        with (
            tc.tile_pool(name="consts", bufs=1) as consts,
            tc.tile_pool(name="io", bufs=8) as io,
            tc.tile_pool(name="ktsb", bufs=4) as ktsb_pool,
            tc.tile_pool(name="vsb", bufs=2 * GROUP + 4) as vsb_pool,
            tc.tile_pool(name="skipsb", bufs=4) as skip_pool,
            tc.tile_pool(name="misc", bufs=10) as misc,
            tc.tile_pool(name="kt_ps", bufs=2, space=MemorySpace.PSUM) as kt_psp,
            tc.tile_pool(name="vs_ps", bufs=2, space=MemorySpace.PSUM) as vs_psp,
            tc.tile_pool(name="sps", bufs=3, space=MemorySpace.PSUM) as sps,
            tc.tile_pool(name="agg_ps", bufs=1, space=MemorySpace.PSUM) as agg_psp,
        ):
            # ---- constants ----
            wk_sb, we_sb, wvs_sb, ct_sb = [], [], [], []
            for dc in range(2):
                dsl = slice(dc * 128, (dc + 1) * 128)
                t = consts.tile([128, F], BF16, tag=f"wk{dc}")
                nc.sync.dma_start(t[:, :], wk_d[dsl, :]); wk_sb.append(t)
                t = consts.tile([128, F], BF16, tag=f"we{dc}")
                nc.sync.dma_start(t[:, :], we_d[dsl, :]); we_sb.append(t)
                t = consts.tile([128, 2 * F], BF16, tag=f"wvs{dc}")
                nc.sync.dma_start(t[:, :], wvs_d[dsl, :]); wvs_sb.append(t)
                t = consts.tile([128, BG], BF16, tag=f"ct{dc}")
                nc.sync.dma_start(t[:, :], ct_d[dsl, :]); ct_sb.append(t)
            qb_sb = []
            for fc, qb_d in enumerate((qb0_d, qb1_d)):
                t = consts.tile([128, BG, 2], BF16, tag=f"qb{fc}")
                nc.sync.dma_start(t[:, :, :], qb_d[:, :, :])
                qb_sb.append(t)
            idb = consts.tile([128, 128], BF16, tag="idb")
            nc.sync.dma_start(idb[:, :], idb_d[:, :])
            idf = consts.tile([128, 128], F32, tag="idf")
            nc.sync.dma_start(idf[:, :], idf_d[:, :])

            # aggregated messages, transposed layout [f_loc, fc, g]; held all
            # kernel. Seeded with the central skip projection Wskip.T @ cT so
            # the per-graph agg matmuls accumulate the final central rows.
            # start=True only on the very first matmul: it clears has_written
            # for the WHOLE bank, so any later start=True here would wipe the
            # other chunk's bits and break accumulation (overwrite semantics).
            agg_ps = agg_psp.tile([128, 2, BG], F32, tag="agg")
            for fc in range(2):
                fsl = slice(F + fc * 128, F + (fc + 1) * 128)
                for dc in range(2):
                    nc.tensor.matmul(agg_ps[:, fc, :], wvs_sb[dc][:, fsl],
                                     ct_sb[dc][:, :],
                                     start=(fc == 0 and dc == 0), stop=False,
                                     skip_group_check=True)

            # ---- main loop over groups of 8 graphs ----
            for grp in range(BG // GROUP):
                g0 = grp * GROUP
                xt8, et8 = [], []
                for dc in range(2):
                    dsl = slice(dc * 128, (dc + 1) * 128)
                    t = io.tile([128, GROUP, N], BF16, tag=f"xt{dc}")
                    nc.sync.dma_start(t[:, :, :],
                                      xt_d[g0:g0 + GROUP, dsl, :].rearrange("g d n -> d g n"))
                    xt8.append(t)
                    t = io.tile([128, GROUP, N], BF16, tag=f"et{dc}")
                    nc.sync.dma_start(t[:, :, :],
                                      et_d[g0:g0 + GROUP, dsl, :].rearrange("g d n -> d g n"))
                    et8.append(t)

                scoresT_ps = sps.tile([128, GROUP * 4], F32, tag="sps")
                v_tiles = []

                for b4 in range(GROUP // NB4):
                    bsl = slice(b4 * NB4, (b4 + 1) * NB4)
                    kt_sb = []
                    for fc in range(2):
                        fsl = slice(fc * 128, (fc + 1) * 128)
                        kt_ps = kt_psp.tile([128, NB4, N], F32, tag="ktps")
                        nc.tensor.matmul(kt_ps[:, :, :], wk_sb[0][:, fsl], xt8[0][:, bsl, :], start=True, stop=False)
                        nc.tensor.matmul(kt_ps[:, :, :], wk_sb[1][:, fsl], xt8[1][:, bsl, :], start=False, stop=False)
                        nc.tensor.matmul(kt_ps[:, :, :], we_sb[0][:, fsl], et8[0][:, bsl, :], start=False, stop=False)
                        nc.tensor.matmul(kt_ps[:, :, :], we_sb[1][:, fsl], et8[1][:, bsl, :], start=False, stop=True)
                        kt = ktsb_pool.tile([128, NB4, N], BF16, tag="ktsb")
                        nc.vector.tensor_copy(kt[:, :, :], kt_ps[:, :, :])
                        kt_sb.append(kt)

                    for gl in range(NB4):
                        gg = b4 * NB4 + gl          # graph index within group
                        g = g0 + gg                 # graph index within core
                        # scoresT: [128 n, 2] per f-chunk
                        for fc in range(2):
                            nc.tensor.matmul(
                                scoresT_ps[:, gg * 4 + fc * 2: gg * 4 + fc * 2 + 2],
                                kt_sb[fc][:, gl, :], qb_sb[fc][:, g, :],
                                start=True, stop=True)
                        # v (+edge) and neighbor skip rows in one [128, 512] psum
                        vs_ps = vs_psp.tile([128, 2 * F], F32, tag="vsps")
                        nc.tensor.matmul(vs_ps[:, :], xt8[0][:, gg, :], wvs_sb[0][:, :], start=True, stop=False)
                        nc.tensor.matmul(vs_ps[:, :], xt8[1][:, gg, :], wvs_sb[1][:, :], start=False, stop=False)
                        nc.tensor.matmul(vs_ps[:, 0:F], et8[0][:, gg, :], we_sb[0][:, :],
                                         start=False, stop=False, skip_group_check=True)
                        nc.tensor.matmul(vs_ps[:, 0:F], et8[1][:, gg, :], we_sb[1][:, :],
                                         start=False, stop=True, skip_group_check=True)
                        v_sb = vsb_pool.tile([128, F], BF16, tag="vsb")
                        nc.vector.tensor_copy(v_sb[:, :], vs_ps[:, 0:F])
                        v_tiles.append(v_sb)
                        skip_sb = skip_pool.tile([128, F], F32, tag="skipsb")
                        # tiny prefix write: absorbs the WAR-on-out-DMA wait so
                        # the big copy below carries only the PE wait (ISA
                        # allows one sync wait per instruction)
                        nc.scalar.activation(skip_sb[0:1, 0:1], idf[0:1, 0:1], AFT.Copy)
                        nc.scalar.activation(skip_sb[:, :], vs_ps[:, F:2 * F], AFT.Copy)
                        nc.sync.dma_start(out_rows[g, 1:ROWS, :], skip_sb[:, :])

                # ---- packed softmax over the group: rows = (g_local, h) ----
                st_sb = misc.tile([128, GROUP * 4], F32, tag="stsb")
                nc.vector.tensor_copy(st_sb[:, :], scoresT_ps[:, :])
                strans_ps = sps.tile([GROUP * 4, 128], F32, tag="sps")
                nc.tensor.transpose(strans_ps[:, :], st_sb[:, :], idf[:, :])
                mx = misc.tile([GROUP * 4, 1], F32, tag="mx")
                nc.vector.reduce_max(mx[:, :], strans_ps[:, :], axis=mybir.AxisListType.X)
                nmx = misc.tile([GROUP * 4, 1], F32, tag="nmx")
                # negate on ScalarE so exp's bias dep is same-engine
                nc.scalar.activation(nmx[:, :], mx[:, :], AFT.Copy, scale=-1.0)
                alpha_sb = misc.tile([GROUP * 4, 128], BF16, tag="alpha")
                sumexp = misc.tile([GROUP * 4, 1], F32, tag="sumexp")
                nc.scalar.activation(alpha_sb[:, :], strans_ps[:, :], AFT.Exp,
                                     bias=nmx[:, 0:1], accum_out=sumexp[:, 0:1])
                rsum = misc.tile([GROUP * 4, 1], F32, tag="rsum")
                nc.vector.reciprocal(rsum[:, :], sumexp[:, :])
                nc.vector.tensor_scalar_mul(alpha_sb[:, :], alpha_sb[:, :], rsum[:, 0:1])
                alphaT_ps = sps.tile([128, GROUP * 4], BF16, tag="sps")
                nc.tensor.transpose(alphaT_ps[:, :], alpha_sb[:, :],
                                    idb[0:GROUP * 4, 0:GROUP * 4])
                alphaT_sb = misc.tile([128, GROUP * 4], BF16, tag="alphaT")
                nc.vector.tensor_copy(alphaT_sb[:, :], alphaT_ps[:, :])

                # ---- aggregate: 4 tiny matmuls per graph, accumulate on the
                # skip-seeded psum ----
                for gg in range(GROUP):
                    g = g0 + gg
                    v_sb = v_tiles[gg]
                    for fc in range(2):
                        for hh in range(2):
                            h = fc * 2 + hh
                            last = (g == BG - 1 and fc == 1 and hh == 1)
                            nc.tensor.matmul(
                                agg_ps[hh * 64:(hh + 1) * 64, fc, g:g + 1],
                                v_sb[:, fc * 128 + hh * 64: fc * 128 + (hh + 1) * 64],
                                alphaT_sb[:, gg * 4 + h: gg * 4 + h + 1],
                                start=False, stop=last, skip_group_check=True)

            # ---- central rows: transpose agg back to [g, f] ----
            cenT_sb = misc.tile([128, 2, BG], F32, tag="cenT")
            nc.vector.tensor_copy(cenT_sb[:, :, :], agg_ps[:, :, :])
            cen_sb = misc.tile([128, F], F32, tag="censb")
            for fc in range(2):
                ct_ps = sps.tile([128, 128], F32, tag="sps")
                nc.tensor.transpose(ct_ps[:, :], cenT_sb[:, fc, :], idf[:, :])
                nc.vector.tensor_copy(cen_sb[:, fc * 128:(fc + 1) * 128], ct_ps[:, :])
            nc.sync.dma_start(out_rows[:, 0, :], cen_sb[:, :])

    nc.compile()
    return nc


def kernel(**inputs):
    x = np.asarray(inputs["neighbor_node_features"], dtype=np.float32)   # [B, N, D]
    e = np.asarray(inputs["edge_features"], dtype=np.float32)            # [B, N, D]
    cen = np.asarray(inputs["central_node_features"], dtype=np.float32)  # [B, 1, D]
    Wq = np.asarray(inputs["Wq"], dtype=np.float32)
    Wk = np.asarray(inputs["Wk"], dtype=np.float32)
    Wv = np.asarray(inputs["Wv"], dtype=np.float32)
    We = np.asarray(inputs["We"], dtype=np.float32)
    Ws = np.asarray(inputs["Wskip"], dtype=np.float32)
    bq = np.asarray(inputs["bq"], dtype=np.float32)
    # biases are all zeros in this model family (bq folds into q host-side)
    for bn in ("bk", "bv", "bskip"):
        bv = np.asarray(inputs[bn])
        assert np.abs(bv).max() == 0.0, f"nonzero bias {bn} unsupported"

    bf = ml_dtypes.bfloat16
    xT = np.ascontiguousarray(x.transpose(0, 2, 1)).astype(bf)    # [B, D, N]
    eT = np.ascontiguousarray(e.transpose(0, 2, 1)).astype(bf)    # [B, D, N]
    cT = cen.reshape(B, D).T                                      # [D, B] f32
    wk = Wk.astype(bf)
    we = We.astype(bf)
    wvs = np.concatenate([Wv, Ws], axis=1).astype(bf)             # [D, 512]

    # host-side q projection + scaling + Qblock masking (tiny GEMM)
    qT = (Wq.T @ cT + bq[:, None]) * (1.0 / np.sqrt(C))           # [F, B] f32
    mask = (np.arange(128) // 64)[:, None] == np.arange(2)[None, :]   # [128, 2]
    qb = np.empty((2, 128, B, 2), dtype=np.float32)
    for fc in range(2):
        qb[fc] = qT[fc * 128:(fc + 1) * 128, :, None] * mask[:, None, :]
    qb = qb.astype(bf)

    idb = np.eye(128, dtype=np.float32).astype(bf)
    idf = np.eye(128, dtype=np.float32)

    if "nc" not in _cached:
        _cached["nc"] = _build_nc()
    nc = _cached["nc"]

    ctb = cT.astype(bf)
    in_maps = []
    for c in range(NCORES):
        gsl = slice(c * BG, (c + 1) * BG)
        in_maps.append({
            "xt": xT[gsl], "et": eT[gsl],
            "ct": np.ascontiguousarray(ctb[:, gsl]),
            "qb0": np.ascontiguousarray(qb[0][:, gsl]),
            "qb1": np.ascontiguousarray(qb[1][:, gsl]),
            "wk": wk, "we": we, "wvs": wvs,
            "idb": idb, "idf": idf,
        })

    from concourse.bass_utils import run_bass_kernel_spmd
    res = run_bass_kernel_spmd(nc, in_maps, core_ids=list(range(NCORES)),
                               **_cached.get("run_kwargs", {}))
    _cached["last_results"] = res
    out = np.concatenate([np.asarray(r["out"]) for r in res.results], axis=0)
    return out.astype(np.float32)



# revision 2
# speedup vs baseline: 1.0695x; 1.0695x over previous
"""Trainium2 Bass kernel for GraphTransformerNet (star-graph TransformerConv).

v8: kT-free fp8 DoubleRow design, pipelined. B=1024 graphs, N=128 neighbors,
D=256, H=4 x C=64 = F=256. Data-parallel over 8 NeuronCores (128 graphs/core).

- scores[n,h] = x.(Wk q) + e.(We q): q folded into per-graph fp8 weight
  vectors kq/eq on host; scores accumulate in a group-lifetime psum bank.
- v = x@Wv + e@We fp8e4m3 DoubleRow (K=256/matmul), weights x8 (descaled
  via the I/8 alphaT identity); kq/eq x16 (descaled inside softmax exp).
- skip projection bf16; v/skip psum banks hold graph PAIRS; skip rows DMA'd
  in graph QUADS via the GpSimd queue (pairs for the last group's tail).
- agg: one matmul per graph (alphaT 4 cols stationary x v moving) ->
  out4[4,256]; host extracts per-head diagonals and adds the f32 central
  skip row. out4 scratch DMA'd every 2 groups.
- group-level software pipelining: group i's alphaT/out4 matmuls are emitted
  AFTER group i+1's projection stream so the in-order PE queue never parks
  on the softmax engine chain; io rings capped at 3 groups so the DMA queues
  aren't flooded at the head; group-0 inputs split into pair-slice DMAs.
"""

import sys

import numpy as np

for _p in ("/opt/trn_rl_repo",):
    if _p not in sys.path:
        sys.path.insert(0, _p)

import ml_dtypes

import concourse.bacc as bacc
import concourse.bass as bass
import concourse.mybir as mybir
from concourse.bass import MemorySpace
from concourse.tile import TileContext

BF16 = mybir.dt.bfloat16
F32 = mybir.dt.float32
FP8 = mybir.dt.float8e4
AFT = mybir.ActivationFunctionType
DR = mybir.MatmulPerfMode.DoubleRow

B, N, D, H, C = 1024, 128, 256, 4, 64
F = H * C            # 256
NCORES = 8
BG = B // NCORES     # 128 graphs per core
GROUP = 8            # graphs per softmax pack
NG = BG // GROUP     # 16 groups per core
ROWS = N + 1         # 129 output rows per graph
WS = 8.0             # fp8 weight pre-scale (descaled via alphaT identity)
QS = 16.0            # kq/eq pre-scale (descaled inside softmax exp)

_cached = {}


def _build_nc():
    nc = bacc.Bacc()

    xt_d = nc.dram_tensor("xt", [NG, 2, 128, GROUP, N], BF16, kind="ExternalInput")
    x2_d = nc.dram_tensor("x2", [NG, 128, 2, GROUP, N], FP8, kind="ExternalInput")
    e2_d = nc.dram_tensor("e2", [NG, 128, 2, GROUP, N], FP8, kind="ExternalInput")
    kq2_d = nc.dram_tensor("kq2", [128, 2, BG, H], FP8, kind="ExternalInput")
    eq2_d = nc.dram_tensor("eq2", [128, 2, BG, H], FP8, kind="ExternalInput")
    wv2_d = nc.dram_tensor("wv2", [128, 2, F], FP8, kind="ExternalInput")
    we2_d = nc.dram_tensor("we2", [128, 2, F], FP8, kind="ExternalInput")
    ws_d = nc.dram_tensor("ws", [2, 128, F], BF16, kind="ExternalInput")
    idb8_d = nc.dram_tensor("idb8", [32, 32], BF16, kind="ExternalInput")
    idf_d = nc.dram_tensor("idf", [128, 128], F32, kind="ExternalInput")
    out_d = nc.dram_tensor("out", [BG * ROWS, F], BF16, kind="ExternalOutput")
    out4_d = nc.dram_tensor("out4", [H, BG, F], BF16, kind="ExternalOutput")

    out_rows = out_d[:, :].rearrange("(g r) f -> g r f", r=ROWS)

    with TileContext(nc) as tc:
        with (
            tc.tile_pool(name="consts", bufs=1) as consts,
            tc.tile_pool(name="io", bufs=3) as io,
            tc.tile_pool(name="vsb", bufs=10) as vsb_pool,
            tc.tile_pool(name="skipsb", bufs=3) as skip_sb_pool,
            tc.tile_pool(name="misc", bufs=3) as misc,
            tc.tile_pool(name="scratch", bufs=1) as scratch_pool,
            tc.tile_pool(name="vs_ps", bufs=2, space=MemorySpace.PSUM) as vs_psp,
            tc.tile_pool(name="sk_ps", bufs=2, space=MemorySpace.PSUM) as sk_psp,
            tc.tile_pool(name="sc_sp", bufs=2, space=MemorySpace.PSUM) as sc_sp,
            tc.tile_pool(name="o4_ps", bufs=2, space=MemorySpace.PSUM) as o4_psp,
        ):
            # ---- weights needed by the very first matmuls ----
            wv2 = consts.tile([128, 2, F], FP8, tag="wv2")
            nc.sync.dma_start(wv2[:, :, :], wv2_d[:, :, :])
            we2 = consts.tile([128, 2, F], FP8, tag="we2")
            nc.sync.dma_start(we2[:, :, :], we2_d[:, :, :])

            def load_group(grp):
                x28 = io.tile([128, 2, GROUP, N], FP8, tag="x2")
                e28 = io.tile([128, 2, GROUP, N], FP8, tag="e2")
                xt8 = [io.tile([128, GROUP, N], BF16, tag=f"xt{dc}",
                               name=f"xt8_{dc}")
                       for dc in range(2)]
                if grp == 0:
                    # split the critical first loads across queues/regions so
                    # the first matmuls start as soon as a pair lands
                    for qq in range(4):
                        gg2 = slice(2 * qq, 2 * qq + 2)
                        nc.sync.dma_start(x28[:, :, gg2, :], x2_d[grp, :, :, gg2, :])
                    for qq in range(4):
                        gg2 = slice(2 * qq, 2 * qq + 2)
                        nc.sync.dma_start(e28[:, :, gg2, :], e2_d[grp, :, :, gg2, :])
                else:
                    nc.sync.dma_start(x28[:, :, :, :], x2_d[grp])
                    nc.sync.dma_start(e28[:, :, :, :], e2_d[grp])
                for dc in range(2):
                    if grp == 0:
                        for hh in range(2):
                            g4 = slice(4 * hh, 4 * hh + 4)
                            nc.sync.dma_start(xt8[dc][:, g4, :], xt_d[grp, dc][:, g4, :])
                    else:
                        nc.sync.dma_start(xt8[dc][:, :, :], xt_d[grp, dc])
                return x28, e28, xt8

            tiles0 = load_group(0)

            # ---- remaining constants (after the group-0 critical loads) ----
            kq2 = consts.tile([128, 2, BG, H], FP8, tag="kq2")
            nc.sync.dma_start(kq2[:, :, :, :], kq2_d[:, :, :, :])
            eq2 = consts.tile([128, 2, BG, H], FP8, tag="eq2")
            nc.sync.dma_start(eq2[:, :, :, :], eq2_d[:, :, :, :])
            ws_sb = []
            for dc in range(2):
                t = consts.tile([128, F], BF16, tag=f"ws{dc}")
                nc.sync.dma_start(t[:, :], ws_d[dc, :, :]); ws_sb.append(t)
            idb8 = consts.tile([32, 32], BF16, tag="idb8")
            nc.sync.dma_start(idb8[:, :], idb8_d[:, :])
            idf = consts.tile([128, 128], F32, tag="idf")
            nc.sync.dma_start(idf[:, :], idf_d[:, :])

            # central-message scratch, DMA'd out every 2 groups
            o4acc = scratch_pool.tile([H, BG, F], BF16, tag="o4acc")

            def emit_group_stream(grp, tiles):
                """Projection matmuls + copies + skip DMAs + softmax engine
                chain for one group. Returns what the deferred agg needs."""
                x28, e28, xt8 = tiles
                g0 = grp * GROUP
                last = grp == NG - 1
                scT_ps = sc_sp.tile([128, GROUP * 4], F32, tag="scsp")
                v_pairs = []
                vs_ps = sk_ps = skip_quad = None

                for gg in range(GROUP):
                    g = g0 + gg
                    j = gg % 2
                    ab = (gg // 2) % 2  # which half of the skip quad
                    if j == 0:
                        vs_ps = vs_psp.tile([128, 2, F], F32, tag="vsps")
                        sk_ps = sk_psp.tile([128, 2, F], F32, tag="skps")
                    nc.tensor.matmul(vs_ps[:, j, :], x28[:, :, gg, :],
                                     wv2[:, :, :], start=(j == 0), stop=False,
                                     perf_mode=DR, skip_group_check=(j == 1))
                    nc.tensor.matmul(vs_ps[:, j, :], e28[:, :, gg, :],
                                     we2[:, :, :], start=False, stop=(j == 1),
                                     perf_mode=DR, skip_group_check=(j == 1))
                    nc.tensor.matmul(scT_ps[:, gg * 4:gg * 4 + 4],
                                     x28[:, :, gg, :], kq2[:, :, g, :],
                                     start=(gg == 0), stop=False, perf_mode=DR,
                                     skip_group_check=(gg > 0))
                    nc.tensor.matmul(scT_ps[:, gg * 4:gg * 4 + 4],
                                     e28[:, :, gg, :], eq2[:, :, g, :],
                                     start=False, stop=(gg == GROUP - 1),
                                     perf_mode=DR, skip_group_check=True)
                    nc.tensor.matmul(sk_ps[:, j, :], xt8[0][:, gg, :],
                                     ws_sb[0][:, :], start=(j == 0), stop=False,
                                     skip_group_check=(j == 1))
                    nc.tensor.matmul(sk_ps[:, j, :], xt8[1][:, gg, :],
                                     ws_sb[1][:, :], start=False, stop=(j == 1),
                                     skip_group_check=(j == 1))

                    if j == 1:
                        v_sb = vsb_pool.tile([128, 2, F], BF16, tag="vsb")
                        if last:
                            # tail: per-pair skip DMAs so the final rows land
                            # as early as possible
                            skip_pair = skip_sb_pool.tile([128, 2, F], BF16,
                                                          tag="skpair", bufs=2)
                            if ab == 0:
                                nc.scalar.activation(skip_pair[0:1, 0, 0:1],
                                                     idf[0:1, 0:1], AFT.Copy)
                                nc.scalar.activation(v_sb[:, :, :],
                                                     vs_ps[:, :, :], AFT.Copy)
                                nc.vector.tensor_copy(skip_pair[:, :, :],
                                                      sk_ps[:, :, :])
                            else:
                                nc.vector.tensor_copy(v_sb[:, :, :],
                                                      vs_ps[:, :, :])
                                nc.scalar.activation(skip_pair[:, :, :],
                                                     sk_ps[:, :, :], AFT.Copy)
                            eng = nc.gpsimd if ab == 0 else nc.sync
                            eng.dma_start(
                                out_rows[g - 1:g + 1, 1:ROWS, :]
                                .rearrange("j n f -> n j f"),
                                skip_pair[:, :, :])
                        elif ab == 0:
                            skip_quad = skip_sb_pool.tile([128, 4, F], BF16,
                                                          tag="skipsb")
                            nc.scalar.activation(skip_quad[0:1, 0, 0:1],
                                                 idf[0:1, 0:1], AFT.Copy)
                            nc.scalar.activation(v_sb[:, :, :], vs_ps[:, :, :],
                                                 AFT.Copy)
                            nc.vector.tensor_copy(skip_quad[:, 0:2, :],
                                                  sk_ps[:, :, :])
                        else:
                            nc.vector.tensor_copy(v_sb[:, :, :], vs_ps[:, :, :])
                            nc.scalar.activation(skip_quad[:, 2:4, :],
                                                 sk_ps[:, :, :], AFT.Copy)
                            nc.gpsimd.dma_start(
                                out_rows[g - 3:g + 1, 1:ROWS, :]
                                .rearrange("j n f -> n j f"),
                                skip_quad[:, :, :])
                        v_pairs.append(v_sb)

                # packed softmax over the group: rows = (g_local, h)
                st_sb = misc.tile([128, GROUP * 4], F32, tag="stsb")
                nc.vector.tensor_copy(st_sb[:, :], scT_ps[:, :])
                strans_ps = sc_sp.tile([GROUP * 4, 128], F32, tag="scsp")
                nc.tensor.transpose(strans_ps[:, :], st_sb[:, :], idf[:, :])
                mx = misc.tile([GROUP * 4, 1], F32, tag="mx")
                nc.vector.reduce_max(mx[:, :], strans_ps[:, :], axis=mybir.AxisListType.X)
                nmx = misc.tile([GROUP * 4, 1], F32, tag="nmx")
                nc.scalar.activation(nmx[:, :], mx[:, :], AFT.Copy, scale=-1.0 / QS)
                alpha_sb = misc.tile([GROUP * 4, 128], BF16, tag="alpha")
                sumexp = misc.tile([GROUP * 4, 1], F32, tag="sumexp")
                nc.scalar.activation(alpha_sb[:, :], strans_ps[:, :], AFT.Exp,
                                     scale=1.0 / QS,
                                     bias=nmx[:, 0:1], accum_out=sumexp[:, 0:1])
                rsum = misc.tile([GROUP * 4, 1], F32, tag="rsum")
                nc.vector.reciprocal(rsum[:, :], sumexp[:, :])
                nc.vector.tensor_scalar_mul(alpha_sb[:, :], alpha_sb[:, :], rsum[:, 0:1])
                return g0, alpha_sb, v_pairs

            def emit_group_agg(pending):
                """alphaT + out4 matmuls, deferred one group for PE density."""
                g0, alpha_sb, v_pairs = pending
                grp = g0 // GROUP
                alphaT_ps = sc_sp.tile([128, GROUP * 4], F32, tag="scsp")
                nc.tensor.matmul(alphaT_ps[:, :], alpha_sb[:, :],
                                 idb8[0:GROUP * 4, 0:GROUP * 4],
                                 start=True, stop=True)
                alphaT_sb = misc.tile([128, GROUP * 4], BF16, tag="alphaT")
                nc.vector.tensor_copy(alphaT_sb[:, :], alphaT_ps[:, :])
                o4_ps = None
                for gg in range(GROUP):
                    j = gg % 2
                    if j == 0:
                        o4_ps = o4_psp.tile([H, 2, F], F32, tag="o4ps")
                    nc.tensor.matmul(o4_ps[:, j, :],
                                     alphaT_sb[:, gg * 4:gg * 4 + 4],
                                     v_pairs[gg // 2][:, j, :],
                                     start=(j == 0), stop=(j == 1),
                                     skip_group_check=(j == 1))
                    if j == 1:
                        if gg % 4 == 1:
                            nc.vector.tensor_copy(
                                o4acc[:, g0 + gg - 1:g0 + gg + 1, :], o4_ps[:, :, :])
                        else:
                            nc.scalar.activation(
                                o4acc[:, g0 + gg - 1:g0 + gg + 1, :], o4_ps[:, :, :],
                                AFT.Copy)
                if grp % 2 == 1:
                    qs = slice((grp - 1) * GROUP, (grp + 1) * GROUP)
                    nc.gpsimd.dma_start(out4_d[:, qs, :], o4acc[:, qs, :])

            # ---- main pipelined loop ----
            pending = None
            for grp in range(NG):
                tiles = tiles0 if grp == 0 else load_group(grp)
                stash = emit_group_stream(grp, tiles)
                if pending is not None:
                    emit_group_agg(pending)
                pending = stash
            emit_group_agg(pending)

    nc.compile()
    return nc


def kernel(**inputs):
    x = np.asarray(inputs["neighbor_node_features"], dtype=np.float32)   # [B, N, D]
    e = np.asarray(inputs["edge_features"], dtype=np.float32)            # [B, N, D]
    cen = np.asarray(inputs["central_node_features"], dtype=np.float32)  # [B, 1, D]
    Wq = np.asarray(inputs["Wq"], dtype=np.float32)
    Wk = np.asarray(inputs["Wk"], dtype=np.float32)
    Wv = np.asarray(inputs["Wv"], dtype=np.float32)
    We = np.asarray(inputs["We"], dtype=np.float32)
    Ws = np.asarray(inputs["Wskip"], dtype=np.float32)
    bq = np.asarray(inputs["bq"], dtype=np.float32)
    # biases are all zeros in this model family (bq folds into q host-side)
    for bn in ("bk", "bv", "bskip"):
        bv = np.asarray(inputs[bn])
        assert np.abs(bv).max() == 0.0, f"nonzero bias {bn} unsupported"

    bf = ml_dtypes.bfloat16
    f8 = ml_dtypes.float8_e4m3
    cen2 = cen.reshape(B, D)
    xT = np.ascontiguousarray(x.transpose(0, 2, 1))               # [B, D, N] f32
    eT = np.ascontiguousarray(e.transpose(0, 2, 1))               # [B, D, N] f32

    # group-major host layouts: 2KB contiguous per partition per group DMA
    def pack_fp8(aT):  # [B, D, N] -> [B/8, 128, 2, 8, N] fp8 (d = i*128 + p)
        a = aT.reshape(B // GROUP, GROUP, 2, 128, N)
        return np.ascontiguousarray(a.transpose(0, 3, 2, 1, 4)).astype(f8)

    x2 = pack_fp8(xT)
    e2 = pack_fp8(eT)
    xtp = np.ascontiguousarray(
        xT.reshape(B // GROUP, GROUP, 2, 128, N).transpose(0, 2, 3, 1, 4)
    ).astype(bf)           # [B/8, 2, 128, 8, N]

    def w2(W):  # [D, F] -> [128, 2, F] fp8, pre-scaled
        return np.ascontiguousarray(
            (W * WS).reshape(2, 128, F).transpose(1, 0, 2)).astype(f8)

    wv2, we2 = w2(Wv), w2(We)
    ws2 = np.ascontiguousarray(Ws.reshape(2, 128, F)).astype(bf)  # [2, 128, F]

    # host: q folded into Wk/We -> per-graph per-head weight vectors kq/eq
    q = (cen2 @ Wq + bq[None, :]) / np.sqrt(C)                    # [B, F]
    kq = np.empty((B, D, H), np.float32)
    eq = np.empty((B, D, H), np.float32)
    for h in range(H):
        qs = q[:, h * C:(h + 1) * C]                              # [B, C]
        kq[:, :, h] = qs @ Wk[:, h * C:(h + 1) * C].T
        eq[:, :, h] = qs @ We[:, h * C:(h + 1) * C].T

    def kq2pack(a):  # [B, D, H] -> [128, 2, B, H] fp8 x QS
        return np.ascontiguousarray(
            (a * QS).reshape(B, 2, 128, H).transpose(2, 1, 0, 3)).astype(f8)

    kq2, eq2 = kq2pack(kq), kq2pack(eq)

    idb8 = (np.eye(32, dtype=np.float32) / WS).astype(bf)
    idf = np.eye(128, dtype=np.float32)

    if "nc" not in _cached:
        _cached["nc"] = _build_nc()
    nc = _cached["nc"]

    in_maps = []
    for c in range(NCORES):
        gsl = slice(c * BG, (c + 1) * BG)
        grsl = slice(c * NG, (c + 1) * NG)
        in_maps.append({
            "xt": np.ascontiguousarray(xtp[grsl]),
            "x2": np.ascontiguousarray(x2[grsl]),
            "e2": np.ascontiguousarray(e2[grsl]),
            "kq2": np.ascontiguousarray(kq2[:, :, gsl, :]),
            "eq2": np.ascontiguousarray(eq2[:, :, gsl, :]),
            "wv2": wv2, "we2": we2, "ws": ws2,
            "idb8": idb8, "idf": idf,
        })

    from concourse.bass_utils import run_bass_kernel_spmd
    res = run_bass_kernel_spmd(nc, in_maps, core_ids=list(range(NCORES)),
                               **_cached.get("run_kwargs", {}))
    _cached["last_results"] = res

    # host epilogue: central rows = c @ Wskip + per-head diagonal of out4
    skip_c = cen2 @ Ws                                            # [B, F] f32
    out = np.empty((B, ROWS, F), np.float32)
    for c in range(NCORES):
        r = res.results[c]
        gsl = slice(c * BG, (c + 1) * BG)
        out[gsl] = np.asarray(r["out"]).reshape(BG, ROWS, F).astype(np.float32)
        o4 = np.asarray(r["out4"]).astype(np.float32)             # [H, BG, F]
        agg = np.concatenate(
            [o4[h, :, h * C:(h + 1) * C] for h in range(H)], axis=1)  # [BG, F]
        out[gsl, 0, :] = skip_c[gsl] + agg
    return out.reshape(B * ROWS, F)


# revision 3
# speedup vs baseline: 1.0713x; 1.0017x over previous
"""Trainium2 Bass kernel for GraphTransformerNet (star-graph TransformerConv).

v9: kT-free fp8 DoubleRow design, pipelined, LDW-balanced. B=1024 graphs, N=128 neighbors,
D=256, H=4 x C=64 = F=256. Data-parallel over 8 NeuronCores (128 graphs/core).

- scores[n,h] = x.(Wk q) + e.(We q): q folded into per-graph fp8 weight
  vectors kq/eq on host; scores accumulate in a group-lifetime psum bank.
- v = x@Wv + e@We fp8e4m3 DoubleRow (K=256/matmul), weights x8 (descaled
  via the I/8 alphaT identity); kq/eq x16 (descaled inside softmax exp).
- skip projection bf16; v/skip psum banks hold graph PAIRS; skip rows DMA'd
  in graph QUADS via the GpSimd queue (pairs for the last group's tail).
- agg: one matmul per graph (alphaT 4 cols stationary x v moving) ->
  out4[4,256]; host extracts per-head diagonals and adds the f32 central
  skip row. out4 scratch DMA'd every 2 groups.
- group-level software pipelining: group i's alphaT/out4 matmuls are emitted
  AFTER group i+1's projection stream so the in-order PE queue never parks
  on the softmax engine chain; io rings capped at 3 groups so the DMA queues
  aren't flooded at the head; group-0 inputs split into pair-slice DMAs.
"""

import sys

import numpy as np

for _p in ("/opt/trn_rl_repo",):
    if _p not in sys.path:
        sys.path.insert(0, _p)

import ml_dtypes

import concourse.bacc as bacc
import concourse.bass as bass
import concourse.mybir as mybir
from concourse.bass import MemorySpace
from concourse.tile import TileContext

BF16 = mybir.dt.bfloat16
F32 = mybir.dt.float32
FP8 = mybir.dt.float8e4
AFT = mybir.ActivationFunctionType
DR = mybir.MatmulPerfMode.DoubleRow

B, N, D, H, C = 1024, 128, 256, 4, 64
F = H * C            # 256
NCORES = 8
BG = B // NCORES     # 128 graphs per core
GROUP = 8            # graphs per softmax pack
NG = BG // GROUP     # 16 groups per core
ROWS = N + 1         # 129 output rows per graph
WS = 8.0             # fp8 weight pre-scale (descaled via alphaT identity)
QS = 16.0            # kq/eq pre-scale (descaled inside softmax exp)

_cached = {}


def _build_nc():
    nc = bacc.Bacc()

    xt_d = nc.dram_tensor("xt", [NG, 2, 128, GROUP, N], BF16, kind="ExternalInput")
    x2_d = nc.dram_tensor("x2", [NG, 128, 2, GROUP, N], FP8, kind="ExternalInput")
    e2_d = nc.dram_tensor("e2", [NG, 128, 2, GROUP, N], FP8, kind="ExternalInput")
    kq2_d = nc.dram_tensor("kq2", [128, 2, BG, H], FP8, kind="ExternalInput")
    eq2_d = nc.dram_tensor("eq2", [128, 2, BG, H], FP8, kind="ExternalInput")
    wv2_d = nc.dram_tensor("wv2", [128, 2, F], FP8, kind="ExternalInput")
    we2_d = nc.dram_tensor("we2", [128, 2, F], FP8, kind="ExternalInput")
    ws_d = nc.dram_tensor("ws", [2, 128, F], BF16, kind="ExternalInput")
    idb8_d = nc.dram_tensor("idb8", [32, 32], BF16, kind="ExternalInput")
    idf_d = nc.dram_tensor("idf", [128, 128], F32, kind="ExternalInput")
    out_d = nc.dram_tensor("out", [BG * ROWS, F], BF16, kind="ExternalOutput")
    out4_d = nc.dram_tensor("out4", [H, BG, F], BF16, kind="ExternalOutput")

    out_rows = out_d[:, :].rearrange("(g r) f -> g r f", r=ROWS)

    with TileContext(nc) as tc:
        with (
            tc.tile_pool(name="consts", bufs=1) as consts,
            tc.tile_pool(name="io", bufs=3) as io,
            tc.tile_pool(name="vsb", bufs=10) as vsb_pool,
            tc.tile_pool(name="skipsb", bufs=3) as skip_sb_pool,
            tc.tile_pool(name="misc", bufs=3) as misc,
            tc.tile_pool(name="scratch", bufs=1) as scratch_pool,
            tc.tile_pool(name="vs_ps", bufs=2, space=MemorySpace.PSUM) as vs_psp,
            tc.tile_pool(name="sk_ps", bufs=2, space=MemorySpace.PSUM) as sk_psp,
            tc.tile_pool(name="sc_sp", bufs=2, space=MemorySpace.PSUM) as sc_sp,
            tc.tile_pool(name="o4_ps", bufs=2, space=MemorySpace.PSUM) as o4_psp,
        ):
            # ---- weights needed by the very first matmuls ----
            wv2 = consts.tile([128, 2, F], FP8, tag="wv2")
            nc.sync.dma_start(wv2[:, :, :], wv2_d[:, :, :])
            we2 = consts.tile([128, 2, F], FP8, tag="we2")
            nc.sync.dma_start(we2[:, :, :], we2_d[:, :, :])

            def load_group(grp):
                x28 = io.tile([128, 2, GROUP, N], FP8, tag="x2")
                e28 = io.tile([128, 2, GROUP, N], FP8, tag="e2")
                xt8 = [io.tile([128, GROUP, N], BF16, tag=f"xt{dc}",
                               name=f"xt8_{dc}")
                       for dc in range(2)]
                if grp == 0:
                    # split the critical first loads across queues/regions AND
                    # dispatch engines so the first matmuls start ASAP
                    xeng = [nc.sync, nc.sync, nc.scalar, nc.scalar]
                    eeng = [nc.gpsimd, nc.gpsimd, nc.sync, nc.gpsimd]
                    for qq in range(4):
                        gg2 = slice(2 * qq, 2 * qq + 2)
                        xeng[qq].dma_start(x28[:, :, gg2, :], x2_d[grp, :, :, gg2, :])
                        eeng[qq].dma_start(e28[:, :, gg2, :], e2_d[grp, :, :, gg2, :])
                else:
                    nc.sync.dma_start(x28[:, :, :, :], x2_d[grp])
                    nc.sync.dma_start(e28[:, :, :, :], e2_d[grp])
                for dc in range(2):
                    if grp == 0:
                        for hh in range(2):
                            g4 = slice(4 * hh, 4 * hh + 4)
                            eng = (nc.scalar, nc.gpsimd)[hh]
                            eng.dma_start(xt8[dc][:, g4, :], xt_d[grp, dc][:, g4, :])
                    else:
                        nc.sync.dma_start(xt8[dc][:, :, :], xt_d[grp, dc])
                return x28, e28, xt8

            tiles0 = load_group(0)

            # ---- remaining constants (after the group-0 critical loads) ----
            kq2 = consts.tile([128, 2, BG, H], FP8, tag="kq2")
            nc.scalar.dma_start(kq2[:, :, :, :], kq2_d[:, :, :, :])
            eq2 = consts.tile([128, 2, BG, H], FP8, tag="eq2")
            nc.gpsimd.dma_start(eq2[:, :, :, :], eq2_d[:, :, :, :])
            ws_sb = []
            for dc in range(2):
                t = consts.tile([128, F], BF16, tag=f"ws{dc}")
                nc.sync.dma_start(t[:, :], ws_d[dc, :, :]); ws_sb.append(t)
            idb8 = consts.tile([32, 32], BF16, tag="idb8")
            nc.sync.dma_start(idb8[:, :], idb8_d[:, :])
            idf = consts.tile([128, 128], F32, tag="idf")
            nc.sync.dma_start(idf[:, :], idf_d[:, :])

            # central-message scratch, DMA'd out every 2 groups
            o4acc = scratch_pool.tile([H, BG, F], BF16, tag="o4acc")

            def emit_group_stream(grp, tiles):
                """Projection matmuls + copies + skip DMAs + softmax engine
                chain for one group. Returns what the deferred agg needs."""
                x28, e28, xt8 = tiles
                g0 = grp * GROUP
                last = grp == NG - 1
                scT_ps = sc_sp.tile([128, GROUP * 4], F32, tag="scsp")
                v_pairs = []
                vs_ps = sk_ps = skip_quad = None

                for gg in range(GROUP):
                    g = g0 + gg
                    j = gg % 2
                    ab = (gg // 2) % 2  # which half of the skip quad
                    if j == 0:
                        vs_ps = vs_psp.tile([128, 2, F], F32, tag="vsps")
                        sk_ps = sk_psp.tile([128, 2, F], F32, tag="skps")
                    nc.tensor.matmul(vs_ps[:, j, :], x28[:, :, gg, :],
                                     wv2[:, :, :], start=(j == 0), stop=False,
                                     perf_mode=DR, skip_group_check=(j == 1))
                    nc.tensor.matmul(vs_ps[:, j, :], e28[:, :, gg, :],
                                     we2[:, :, :], start=False, stop=(j == 1),
                                     perf_mode=DR, skip_group_check=(j == 1))
                    for dc in range(2):
                        nc.tensor.matmul(scT_ps[:, gg * 4:gg * 4 + 4],
                                         x28[:, dc, gg, :], kq2[:, dc, g, :],
                                         start=(gg == 0 and dc == 0), stop=False,
                                         skip_group_check=not (gg == 0 and dc == 0))
                        nc.tensor.matmul(scT_ps[:, gg * 4:gg * 4 + 4],
                                         e28[:, dc, gg, :], eq2[:, dc, g, :],
                                         start=False,
                                         stop=(gg == GROUP - 1 and dc == 1),
                                         skip_group_check=True)
                    nc.tensor.matmul(sk_ps[:, j, :], xt8[0][:, gg, :],
                                     ws_sb[0][:, :], start=(j == 0), stop=False,
                                     skip_group_check=(j == 1))
                    nc.tensor.matmul(sk_ps[:, j, :], xt8[1][:, gg, :],
                                     ws_sb[1][:, :], start=False, stop=(j == 1),
                                     skip_group_check=(j == 1))

                    if j == 1:
                        v_sb = vsb_pool.tile([128, 2, F], BF16, tag="vsb")
                        if last:
                            # tail: per-pair skip DMAs so the final rows land
                            # as early as possible
                            skip_pair = skip_sb_pool.tile([128, 2, F], BF16,
                                                          tag="skpair", bufs=2)
                            if ab == 0:
                                nc.scalar.activation(skip_pair[0:1, 0, 0:1],
                                                     idf[0:1, 0:1], AFT.Copy)
                                nc.scalar.activation(v_sb[:, :, :],
                                                     vs_ps[:, :, :], AFT.Copy)
                                nc.vector.tensor_copy(skip_pair[:, :, :],
                                                      sk_ps[:, :, :])
                            else:
                                nc.vector.tensor_copy(v_sb[:, :, :],
                                                      vs_ps[:, :, :])
                                nc.scalar.activation(skip_pair[:, :, :],
                                                     sk_ps[:, :, :], AFT.Copy)
                            eng = nc.gpsimd if ab == 0 else nc.sync
                            eng.dma_start(
                                out_rows[g - 1:g + 1, 1:ROWS, :]
                                .rearrange("j n f -> n j f"),
                                skip_pair[:, :, :])
                        elif ab == 0:
                            skip_quad = skip_sb_pool.tile([128, 4, F], BF16,
                                                          tag="skipsb")
                            nc.scalar.activation(skip_quad[0:1, 0, 0:1],
                                                 idf[0:1, 0:1], AFT.Copy)
                            nc.scalar.activation(v_sb[:, :, :], vs_ps[:, :, :],
                                                 AFT.Copy)
                            nc.vector.tensor_copy(skip_quad[:, 0:2, :],
                                                  sk_ps[:, :, :])
                        else:
                            nc.vector.tensor_copy(v_sb[:, :, :], vs_ps[:, :, :])
                            nc.scalar.activation(skip_quad[:, 2:4, :],
                                                 sk_ps[:, :, :], AFT.Copy)
                            nc.gpsimd.dma_start(
                                out_rows[g - 3:g + 1, 1:ROWS, :]
                                .rearrange("j n f -> n j f"),
                                skip_quad[:, :, :])
                        v_pairs.append(v_sb)

                # packed softmax over the group: rows = (g_local, h)
                st_sb = misc.tile([128, GROUP * 4], F32, tag="stsb")
                nc.vector.tensor_copy(st_sb[:, :], scT_ps[:, :])
                strans_ps = sc_sp.tile([GROUP * 4, 128], F32, tag="scsp")
                nc.tensor.transpose(strans_ps[:, :], st_sb[:, :], idf[:, :])
                mx = misc.tile([GROUP * 4, 1], F32, tag="mx")
                nc.vector.reduce_max(mx[:, :], strans_ps[:, :], axis=mybir.AxisListType.X)
                nmx = misc.tile([GROUP * 4, 1], F32, tag="nmx")
                nc.scalar.activation(nmx[:, :], mx[:, :], AFT.Copy, scale=-1.0 / QS)
                alpha_sb = misc.tile([GROUP * 4, 128], BF16, tag="alpha")
                sumexp = misc.tile([GROUP * 4, 1], F32, tag="sumexp")
                nc.scalar.activation(alpha_sb[:, :], strans_ps[:, :], AFT.Exp,
                                     scale=1.0 / QS,
                                     bias=nmx[:, 0:1], accum_out=sumexp[:, 0:1])
                rsum = misc.tile([GROUP * 4, 1], F32, tag="rsum")
                nc.vector.reciprocal(rsum[:, :], sumexp[:, :])
                nc.vector.tensor_scalar_mul(alpha_sb[:, :], alpha_sb[:, :], rsum[:, 0:1])
                return g0, alpha_sb, v_pairs

            def emit_group_agg(pending):
                """alphaT + out4 matmuls, deferred one group for PE density."""
                g0, alpha_sb, v_pairs = pending
                grp = g0 // GROUP
                alphaT_ps = sc_sp.tile([128, GROUP * 4], F32, tag="scsp")
                nc.tensor.matmul(alphaT_ps[:, :], alpha_sb[:, :],
                                 idb8[0:GROUP * 4, 0:GROUP * 4],
                                 start=True, stop=True)
                alphaT_sb = misc.tile([128, GROUP * 4], BF16, tag="alphaT")
                nc.vector.tensor_copy(alphaT_sb[:, :], alphaT_ps[:, :])
                o4_ps = None
                for gg in range(GROUP):
                    j = gg % 2
                    if j == 0:
                        o4_ps = o4_psp.tile([H, 2, F], F32, tag="o4ps")
                    nc.tensor.matmul(o4_ps[:, j, :],
                                     alphaT_sb[:, gg * 4:gg * 4 + 4],
                                     v_pairs[gg // 2][:, j, :],
                                     start=(j == 0), stop=(j == 1),
                                     skip_group_check=(j == 1))
                    if j == 1:
                        if gg % 4 == 1:
                            nc.vector.tensor_copy(
                                o4acc[:, g0 + gg - 1:g0 + gg + 1, :], o4_ps[:, :, :])
                        else:
                            nc.scalar.activation(
                                o4acc[:, g0 + gg - 1:g0 + gg + 1, :], o4_ps[:, :, :],
                                AFT.Copy)
                if grp >= NG - 2:
                    qs = slice(grp * GROUP, (grp + 1) * GROUP)
                    nc.sync.dma_start(out4_d[:, qs, :], o4acc[:, qs, :])
                elif grp % 2 == 1:
                    qs = slice((grp - 1) * GROUP, (grp + 1) * GROUP)
                    nc.gpsimd.dma_start(out4_d[:, qs, :], o4acc[:, qs, :])

            # ---- main pipelined loop ----
            pending = None
            for grp in range(NG):
                tiles = tiles0 if grp == 0 else load_group(grp)
                stash = emit_group_stream(grp, tiles)
                if pending is not None:
                    emit_group_agg(pending)
                pending = stash
            emit_group_agg(pending)

    nc.compile()
    return nc


def kernel(**inputs):
    x = np.asarray(inputs["neighbor_node_features"], dtype=np.float32)   # [B, N, D]
    e = np.asarray(inputs["edge_features"], dtype=np.float32)            # [B, N, D]
    cen = np.asarray(inputs["central_node_features"], dtype=np.float32)  # [B, 1, D]
    Wq = np.asarray(inputs["Wq"], dtype=np.float32)
    Wk = np.asarray(inputs["Wk"], dtype=np.float32)
    Wv = np.asarray(inputs["Wv"], dtype=np.float32)
    We = np.asarray(inputs["We"], dtype=np.float32)
    Ws = np.asarray(inputs["Wskip"], dtype=np.float32)
    bq = np.asarray(inputs["bq"], dtype=np.float32)
    # biases are all zeros in this model family (bq folds into q host-side)
    for bn in ("bk", "bv", "bskip"):
        bv = np.asarray(inputs[bn])
        assert np.abs(bv).max() == 0.0, f"nonzero bias {bn} unsupported"

    bf = ml_dtypes.bfloat16
    f8 = ml_dtypes.float8_e4m3
    cen2 = cen.reshape(B, D)
    xT = np.ascontiguousarray(x.transpose(0, 2, 1))               # [B, D, N] f32
    eT = np.ascontiguousarray(e.transpose(0, 2, 1))               # [B, D, N] f32

    # group-major host layouts: 2KB contiguous per partition per group DMA
    def pack_fp8(aT):  # [B, D, N] -> [B/8, 128, 2, 8, N] fp8 (d = i*128 + p)
        a = aT.reshape(B // GROUP, GROUP, 2, 128, N)
        return np.ascontiguousarray(a.transpose(0, 3, 2, 1, 4)).astype(f8)

    x2 = pack_fp8(xT)
    e2 = pack_fp8(eT)
    xtp = np.ascontiguousarray(
        xT.reshape(B // GROUP, GROUP, 2, 128, N).transpose(0, 2, 3, 1, 4)
    ).astype(bf)           # [B/8, 2, 128, 8, N]

    def w2(W):  # [D, F] -> [128, 2, F] fp8, pre-scaled
        return np.ascontiguousarray(
            (W * WS).reshape(2, 128, F).transpose(1, 0, 2)).astype(f8)

    wv2, we2 = w2(Wv), w2(We)
    ws2 = np.ascontiguousarray(Ws.reshape(2, 128, F)).astype(bf)  # [2, 128, F]

    # host: q folded into Wk/We -> per-graph per-head weight vectors kq/eq
    q = (cen2 @ Wq + bq[None, :]) / np.sqrt(C)                    # [B, F]
    kq = np.empty((B, D, H), np.float32)
    eq = np.empty((B, D, H), np.float32)
    for h in range(H):
        qs = q[:, h * C:(h + 1) * C]                              # [B, C]
        kq[:, :, h] = qs @ Wk[:, h * C:(h + 1) * C].T
        eq[:, :, h] = qs @ We[:, h * C:(h + 1) * C].T

    def kq2pack(a):  # [B, D, H] -> [128, 2, B, H] fp8 x QS
        return np.ascontiguousarray(
            (a * QS).reshape(B, 2, 128, H).transpose(2, 1, 0, 3)).astype(f8)

    kq2, eq2 = kq2pack(kq), kq2pack(eq)

    idb8 = (np.eye(32, dtype=np.float32) / WS).astype(bf)
    idf = np.eye(128, dtype=np.float32)

    if "nc" not in _cached:
        _cached["nc"] = _build_nc()
    nc = _cached["nc"]

    in_maps = []
    for c in range(NCORES):
        gsl = slice(c * BG, (c + 1) * BG)
        grsl = slice(c * NG, (c + 1) * NG)
        in_maps.append({
            "xt": np.ascontiguousarray(xtp[grsl]),
            "x2": np.ascontiguousarray(x2[grsl]),
            "e2": np.ascontiguousarray(e2[grsl]),
            "kq2": np.ascontiguousarray(kq2[:, :, gsl, :]),
            "eq2": np.ascontiguousarray(eq2[:, :, gsl, :]),
            "wv2": wv2, "we2": we2, "ws": ws2,
            "idb8": idb8, "idf": idf,
        })

    from concourse.bass_utils import run_bass_kernel_spmd
    res = run_bass_kernel_spmd(nc, in_maps, core_ids=list(range(NCORES)),
                               **_cached.get("run_kwargs", {}))
    _cached["last_results"] = res

    # host epilogue: central rows = c @ Wskip + per-head diagonal of out4
    skip_c = cen2 @ Ws                                            # [B, F] f32
    out = np.empty((B, ROWS, F), np.float32)
    for c in range(NCORES):
        r = res.results[c]
        gsl = slice(c * BG, (c + 1) * BG)
        out[gsl] = np.asarray(r["out"]).reshape(BG, ROWS, F).astype(np.float32)
        o4 = np.asarray(r["out4"]).astype(np.float32)             # [H, BG, F]
        agg = np.concatenate(
            [o4[h, :, h * C:(h + 1) * C] for h in range(H)], axis=1)  # [BG, F]
        out[gsl, 0, :] = skip_c[gsl] + agg
    return out.reshape(B * ROWS, F)


# revision 4
# speedup vs baseline: 1.0783x; 1.0065x over previous
"""Trainium2 Bass kernel for GraphTransformerNet (star-graph TransformerConv).

v9: kT-free fp8 DoubleRow design, pipelined, LDW-balanced. B=1024 graphs, N=128 neighbors,
D=256, H=4 x C=64 = F=256. Data-parallel over 8 NeuronCores (128 graphs/core).

- scores[n,h] = x.(Wk q) + e.(We q): q folded into per-graph fp8 weight
  vectors kq/eq on host; scores accumulate in a group-lifetime psum bank.
- v = x@Wv + e@We fp8e4m3 DoubleRow (K=256/matmul), weights x8 (descaled
  via the I/8 alphaT identity); kq/eq x16 (descaled inside softmax exp).
- skip projection bf16; v/skip psum banks hold graph PAIRS; skip rows DMA'd
  in graph QUADS via the GpSimd queue (pairs for the last group's tail).
- agg: one matmul per graph (alphaT 4 cols stationary x v moving) ->
  out4[4,256]; host extracts per-head diagonals and adds the f32 central
  skip row. out4 scratch DMA'd every 2 groups.
- group-level software pipelining: group i's alphaT/out4 matmuls are emitted
  AFTER group i+1's projection stream so the in-order PE queue never parks
  on the softmax engine chain; io rings capped at 3 groups so the DMA queues
  aren't flooded at the head; group-0 inputs split into pair-slice DMAs.
"""

import sys

import numpy as np

for _p in ("/opt/trn_rl_repo",):
    if _p not in sys.path:
        sys.path.insert(0, _p)

import ml_dtypes

import concourse.bacc as bacc
import concourse.bass as bass
import concourse.mybir as mybir
from concourse.bass import MemorySpace
from concourse.tile import TileContext

BF16 = mybir.dt.bfloat16
F32 = mybir.dt.float32
FP8 = mybir.dt.float8e4
AFT = mybir.ActivationFunctionType
DR = mybir.MatmulPerfMode.DoubleRow

B, N, D, H, C = 1024, 128, 256, 4, 64
F = H * C            # 256
NCORES = 8
BG = B // NCORES     # 128 graphs per core
GROUP = 8            # graphs per softmax pack
NG = BG // GROUP     # 16 groups per core
ROWS = N + 1         # 129 output rows per graph
WS = 8.0             # fp8 weight pre-scale (descaled via alphaT identity)
QS = 16.0            # kq/eq pre-scale (descaled inside softmax exp)

_cached = {}


def _build_nc():
    nc = bacc.Bacc()

    xt_d = nc.dram_tensor("xt", [NG, 2, 128, GROUP, N], BF16, kind="ExternalInput")
    x2_d = nc.dram_tensor("x2", [NG, 128, 2, GROUP, N], FP8, kind="ExternalInput")
    e2_d = nc.dram_tensor("e2", [NG, 128, 2, GROUP, N], FP8, kind="ExternalInput")
    kq2_d = nc.dram_tensor("kq2", [128, 2, BG, H], FP8, kind="ExternalInput")
    eq2_d = nc.dram_tensor("eq2", [128, 2, BG, H], FP8, kind="ExternalInput")
    wv2_d = nc.dram_tensor("wv2", [128, 2, F], FP8, kind="ExternalInput")
    we2_d = nc.dram_tensor("we2", [128, 2, F], FP8, kind="ExternalInput")
    ws_d = nc.dram_tensor("ws", [2, 128, F], BF16, kind="ExternalInput")
    idb8_d = nc.dram_tensor("idb8", [32, 32], BF16, kind="ExternalInput")
    idf_d = nc.dram_tensor("idf", [128, 128], F32, kind="ExternalInput")
    out_d = nc.dram_tensor("out", [BG * ROWS, F], BF16, kind="ExternalOutput")
    out4_d = nc.dram_tensor("out4", [H, BG, F], BF16, kind="ExternalOutput")

    out_rows = out_d[:, :].rearrange("(g r) f -> g r f", r=ROWS)

    with TileContext(nc) as tc:
        with (
            tc.tile_pool(name="consts", bufs=1) as consts,
            tc.tile_pool(name="io", bufs=3) as io,
            tc.tile_pool(name="vsb", bufs=10) as vsb_pool,
            tc.tile_pool(name="skipsb", bufs=3) as skip_sb_pool,
            tc.tile_pool(name="misc", bufs=3) as misc,
            tc.tile_pool(name="scratch", bufs=1) as scratch_pool,
            tc.tile_pool(name="vs_ps", bufs=2, space=MemorySpace.PSUM) as vs_psp,
            tc.tile_pool(name="sk_ps", bufs=2, space=MemorySpace.PSUM) as sk_psp,
            tc.tile_pool(name="sc_sp", bufs=2, space=MemorySpace.PSUM) as sc_sp,
            tc.tile_pool(name="o4_ps", bufs=2, space=MemorySpace.PSUM) as o4_psp,
        ):
            # ---- weights needed by the very first matmuls ----
            wv2 = consts.tile([128, 2, F], FP8, tag="wv2")
            nc.sync.dma_start(wv2[:, :, :], wv2_d[:, :, :])
            we2 = consts.tile([128, 2, F], FP8, tag="we2")
            nc.sync.dma_start(we2[:, :, :], we2_d[:, :, :])

            def load_group(grp):
                x28 = io.tile([128, 2, GROUP, N], FP8, tag="x2")
                e28 = io.tile([128, 2, GROUP, N], FP8, tag="e2")
                xt8 = [io.tile([128, GROUP, N], BF16, tag=f"xt{dc}",
                               name=f"xt8_{dc}")
                       for dc in range(2)]
                if grp == 0:
                    # split the critical first loads across queues/regions AND
                    # dispatch engines so the first matmuls start ASAP
                    xeng = [nc.sync, nc.sync, nc.scalar, nc.scalar]
                    eeng = [nc.gpsimd, nc.gpsimd, nc.sync, nc.gpsimd]
                    for qq in range(4):
                        gg2 = slice(2 * qq, 2 * qq + 2)
                        xeng[qq].dma_start(x28[:, :, gg2, :], x2_d[grp, :, :, gg2, :])
                        eeng[qq].dma_start(e28[:, :, gg2, :], e2_d[grp, :, :, gg2, :])
                else:
                    nc.sync.dma_start(x28[:, :, :, :], x2_d[grp])
                    nc.sync.dma_start(e28[:, :, :, :], e2_d[grp])
                for dc in range(2):
                    if grp == 0:
                        for hh in range(2):
                            g4 = slice(4 * hh, 4 * hh + 4)
                            eng = (nc.scalar, nc.gpsimd)[hh]
                            eng.dma_start(xt8[dc][:, g4, :], xt_d[grp, dc][:, g4, :])
                    else:
                        nc.sync.dma_start(xt8[dc][:, :, :], xt_d[grp, dc])
                return x28, e28, xt8

            tiles0 = load_group(0)

            # ---- remaining constants (after the group-0 critical loads) ----
            kq2 = consts.tile([128, 2, BG, H], FP8, tag="kq2")
            nc.scalar.dma_start(kq2[:, :, :, :], kq2_d[:, :, :, :])
            eq2 = consts.tile([128, 2, BG, H], FP8, tag="eq2")
            nc.gpsimd.dma_start(eq2[:, :, :, :], eq2_d[:, :, :, :])
            ws_sb = []
            for dc in range(2):
                t = consts.tile([128, F], BF16, tag=f"ws{dc}")
                nc.sync.dma_start(t[:, :], ws_d[dc, :, :]); ws_sb.append(t)
            idb8 = consts.tile([32, 32], BF16, tag="idb8")
            nc.sync.dma_start(idb8[:, :], idb8_d[:, :])
            idf = consts.tile([128, 128], F32, tag="idf")
            nc.sync.dma_start(idf[:, :], idf_d[:, :])

            # central-message scratch, DMA'd out every 2 groups
            o4acc = scratch_pool.tile([H, BG, F], BF16, tag="o4acc")

            def emit_group_stream(grp, tiles):
                """Projection matmuls + copies + skip DMAs + softmax engine
                chain for one group. Returns what the deferred agg needs."""
                x28, e28, xt8 = tiles
                g0 = grp * GROUP
                last = grp == NG - 1
                scT_ps = sc_sp.tile([128, GROUP * 4], F32, tag="scsp")
                v_pairs = []
                vs_ps = sk_ps = skip_quad = None

                for gg in range(GROUP):
                    g = g0 + gg
                    j = gg % 2
                    ab = (gg // 2) % 2  # which half of the skip quad
                    if j == 0:
                        vs_ps = vs_psp.tile([128, 2, F], F32, tag="vsps")
                        sk_ps = sk_psp.tile([128, 2, F], F32, tag="skps")
                    nc.tensor.matmul(vs_ps[:, j, :], x28[:, :, gg, :],
                                     wv2[:, :, :], start=(j == 0), stop=False,
                                     perf_mode=DR, skip_group_check=(j == 1))
                    nc.tensor.matmul(vs_ps[:, j, :], e28[:, :, gg, :],
                                     we2[:, :, :], start=False, stop=(j == 1),
                                     perf_mode=DR, skip_group_check=(j == 1))
                    for dc in range(2):
                        nc.tensor.matmul(scT_ps[:, gg * 4:gg * 4 + 4],
                                         x28[:, dc, gg, :], kq2[:, dc, g, :],
                                         start=(gg == 0 and dc == 0), stop=False,
                                         skip_group_check=not (gg == 0 and dc == 0))
                        nc.tensor.matmul(scT_ps[:, gg * 4:gg * 4 + 4],
                                         e28[:, dc, gg, :], eq2[:, dc, g, :],
                                         start=False,
                                         stop=(gg == GROUP - 1 and dc == 1),
                                         skip_group_check=True)
                    nc.tensor.matmul(sk_ps[:, j, :], xt8[0][:, gg, :],
                                     ws_sb[0][:, :], start=(j == 0), stop=False,
                                     skip_group_check=(j == 1))
                    nc.tensor.matmul(sk_ps[:, j, :], xt8[1][:, gg, :],
                                     ws_sb[1][:, :], start=False, stop=(j == 1),
                                     skip_group_check=(j == 1))

                    if j == 1:
                        v_sb = vsb_pool.tile([128, 2, F], BF16, tag="vsb")
                        if last:
                            # tail: per-pair skip DMAs so the final rows land
                            # as early as possible
                            skip_pair = skip_sb_pool.tile([128, 2, F], BF16,
                                                          tag="skpair", bufs=2)
                            if ab == 0:
                                nc.scalar.activation(skip_pair[0:1, 0, 0:1],
                                                     idf[0:1, 0:1], AFT.Copy)
                                nc.scalar.activation(v_sb[:, :, :],
                                                     vs_ps[:, :, :], AFT.Copy)
                                nc.vector.tensor_copy(skip_pair[:, :, :],
                                                      sk_ps[:, :, :])
                            else:
                                nc.vector.tensor_copy(v_sb[:, :, :],
                                                      vs_ps[:, :, :])
                                nc.scalar.activation(skip_pair[:, :, :],
                                                     sk_ps[:, :, :], AFT.Copy)
                            eng = nc.gpsimd if ab == 0 else nc.sync
                            eng.dma_start(
                                out_rows[g - 1:g + 1, 1:ROWS, :]
                                .rearrange("j n f -> n j f"),
                                skip_pair[:, :, :])
                        elif ab == 0:
                            skip_quad = skip_sb_pool.tile([128, 4, F], BF16,
                                                          tag="skipsb")
                            nc.scalar.activation(skip_quad[0:1, 0, 0:1],
                                                 idf[0:1, 0:1], AFT.Copy)
                            nc.scalar.activation(v_sb[:, :, :], vs_ps[:, :, :],
                                                 AFT.Copy)
                            nc.vector.tensor_copy(skip_quad[:, 0:2, :],
                                                  sk_ps[:, :, :])
                        else:
                            nc.vector.tensor_copy(v_sb[:, :, :], vs_ps[:, :, :])
                            nc.scalar.activation(skip_quad[:, 2:4, :],
                                                 sk_ps[:, :, :], AFT.Copy)
                            nc.gpsimd.dma_start(
                                out_rows[g - 3:g + 1, 1:ROWS, :]
                                .rearrange("j n f -> n j f"),
                                skip_quad[:, :, :])
                        v_pairs.append(v_sb)

                # packed softmax over the group: rows = (g_local, h)
                st_sb = misc.tile([128, GROUP * 4], F32, tag="stsb")
                nc.vector.tensor_copy(st_sb[:, :], scT_ps[:, :])
                strans_ps = sc_sp.tile([GROUP * 4, 128], F32, tag="scsp")
                nc.tensor.transpose(strans_ps[:, :], st_sb[:, :], idf[:, :])
                mx = misc.tile([GROUP * 4, 1], F32, tag="mx")
                nc.vector.reduce_max(mx[:, :], strans_ps[:, :], axis=mybir.AxisListType.X)
                nmx = misc.tile([GROUP * 4, 1], F32, tag="nmx")
                nc.scalar.activation(nmx[:, :], mx[:, :], AFT.Copy, scale=-1.0 / QS)
                alpha_sb = misc.tile([GROUP * 4, 128], BF16, tag="alpha")
                sumexp = misc.tile([GROUP * 4, 1], F32, tag="sumexp")
                nc.scalar.activation(alpha_sb[:, :], strans_ps[:, :], AFT.Exp,
                                     scale=1.0 / QS,
                                     bias=nmx[:, 0:1], accum_out=sumexp[:, 0:1])
                rsum = misc.tile([GROUP * 4, 1], F32, tag="rsum")
                nc.vector.reciprocal(rsum[:, :], sumexp[:, :])
                nc.vector.tensor_scalar_mul(alpha_sb[:, :], alpha_sb[:, :], rsum[:, 0:1])
                return g0, alpha_sb, v_pairs

            def emit_group_agg(pending):
                """alphaT + out4 matmuls, deferred one group for PE density."""
                g0, alpha_sb, v_pairs = pending
                grp = g0 // GROUP
                alphaT_ps = sc_sp.tile([128, GROUP * 4], F32, tag="scsp")
                nc.tensor.matmul(alphaT_ps[:, :], alpha_sb[:, :],
                                 idb8[0:GROUP * 4, 0:GROUP * 4],
                                 start=True, stop=True)
                alphaT_sb = misc.tile([128, GROUP * 4], BF16, tag="alphaT")
                nc.vector.tensor_copy(alphaT_sb[:, :], alphaT_ps[:, :])
                o4_ps = None
                for gg in range(GROUP):
                    j = gg % 2
                    if j == 0:
                        o4_ps = o4_psp.tile([H, 2, F], F32, tag="o4ps")
                    nc.tensor.matmul(o4_ps[:, j, :],
                                     alphaT_sb[:, gg * 4:gg * 4 + 4],
                                     v_pairs[gg // 2][:, j, :],
                                     start=(j == 0), stop=(j == 1),
                                     skip_group_check=(j == 1))
                    if j == 1:
                        psl = slice(g0 + gg - 1, g0 + gg + 1)
                        if gg % 4 == 1:
                            nc.vector.tensor_copy(o4acc[:, psl, :], o4_ps[:, :, :])
                        else:
                            nc.scalar.activation(o4acc[:, psl, :], o4_ps[:, :, :],
                                                 AFT.Copy)
                        if grp >= NG - 2:
                            nc.gpsimd.dma_start(out4_d[:, psl, :], o4acc[:, psl, :])
                if grp >= NG - 2:
                    pass  # per-pair DMAs emitted inline below
                elif grp % 2 == 1:
                    qs = slice((grp - 1) * GROUP, (grp + 1) * GROUP)
                    nc.gpsimd.dma_start(out4_d[:, qs, :], o4acc[:, qs, :])

            # ---- main pipelined loop ----
            pending = None
            for grp in range(NG):
                tiles = tiles0 if grp == 0 else load_group(grp)
                stash = emit_group_stream(grp, tiles)
                if pending is not None:
                    emit_group_agg(pending)
                pending = stash
            emit_group_agg(pending)

    nc.compile()
    return nc


def kernel(**inputs):
    x = np.asarray(inputs["neighbor_node_features"], dtype=np.float32)   # [B, N, D]
    e = np.asarray(inputs["edge_features"], dtype=np.float32)            # [B, N, D]
    cen = np.asarray(inputs["central_node_features"], dtype=np.float32)  # [B, 1, D]
    Wq = np.asarray(inputs["Wq"], dtype=np.float32)
    Wk = np.asarray(inputs["Wk"], dtype=np.float32)
    Wv = np.asarray(inputs["Wv"], dtype=np.float32)
    We = np.asarray(inputs["We"], dtype=np.float32)
    Ws = np.asarray(inputs["Wskip"], dtype=np.float32)
    bq = np.asarray(inputs["bq"], dtype=np.float32)
    # biases are all zeros in this model family (bq folds into q host-side)
    for bn in ("bk", "bv", "bskip"):
        bv = np.asarray(inputs[bn])
        assert np.abs(bv).max() == 0.0, f"nonzero bias {bn} unsupported"

    bf = ml_dtypes.bfloat16
    f8 = ml_dtypes.float8_e4m3
    cen2 = cen.reshape(B, D)
    xT = np.ascontiguousarray(x.transpose(0, 2, 1))               # [B, D, N] f32
    eT = np.ascontiguousarray(e.transpose(0, 2, 1))               # [B, D, N] f32

    # group-major host layouts: 2KB contiguous per partition per group DMA
    def pack_fp8(aT):  # [B, D, N] -> [B/8, 128, 2, 8, N] fp8 (d = i*128 + p)
        a = aT.reshape(B // GROUP, GROUP, 2, 128, N)
        return np.ascontiguousarray(a.transpose(0, 3, 2, 1, 4)).astype(f8)

    x2 = pack_fp8(xT)
    e2 = pack_fp8(eT)
    xtp = np.ascontiguousarray(
        xT.reshape(B // GROUP, GROUP, 2, 128, N).transpose(0, 2, 3, 1, 4)
    ).astype(bf)           # [B/8, 2, 128, 8, N]

    def w2(W):  # [D, F] -> [128, 2, F] fp8, pre-scaled
        return np.ascontiguousarray(
            (W * WS).reshape(2, 128, F).transpose(1, 0, 2)).astype(f8)

    wv2, we2 = w2(Wv), w2(We)
    ws2 = np.ascontiguousarray(Ws.reshape(2, 128, F)).astype(bf)  # [2, 128, F]

    # host: q folded into Wk/We -> per-graph per-head weight vectors kq/eq
    q = (cen2 @ Wq + bq[None, :]) / np.sqrt(C)                    # [B, F]
    kq = np.empty((B, D, H), np.float32)
    eq = np.empty((B, D, H), np.float32)
    for h in range(H):
        qs = q[:, h * C:(h + 1) * C]                              # [B, C]
        kq[:, :, h] = qs @ Wk[:, h * C:(h + 1) * C].T
        eq[:, :, h] = qs @ We[:, h * C:(h + 1) * C].T

    def kq2pack(a):  # [B, D, H] -> [128, 2, B, H] fp8 x QS
        return np.ascontiguousarray(
            (a * QS).reshape(B, 2, 128, H).transpose(2, 1, 0, 3)).astype(f8)

    kq2, eq2 = kq2pack(kq), kq2pack(eq)

    idb8 = (np.eye(32, dtype=np.float32) / WS).astype(bf)
    idf = np.eye(128, dtype=np.float32)

    if "nc" not in _cached:
        _cached["nc"] = _build_nc()
    nc = _cached["nc"]

    in_maps = []
    for c in range(NCORES):
        gsl = slice(c * BG, (c + 1) * BG)
        grsl = slice(c * NG, (c + 1) * NG)
        in_maps.append({
            "xt": np.ascontiguousarray(xtp[grsl]),
            "x2": np.ascontiguousarray(x2[grsl]),
            "e2": np.ascontiguousarray(e2[grsl]),
            "kq2": np.ascontiguousarray(kq2[:, :, gsl, :]),
            "eq2": np.ascontiguousarray(eq2[:, :, gsl, :]),
            "wv2": wv2, "we2": we2, "ws": ws2,
            "idb8": idb8, "idf": idf,
        })

    from concourse.bass_utils import run_bass_kernel_spmd
    res = run_bass_kernel_spmd(nc, in_maps, core_ids=list(range(NCORES)),
                               **_cached.get("run_kwargs", {}))
    _cached["last_results"] = res

    # host epilogue: central rows = c @ Wskip + per-head diagonal of out4
    skip_c = cen2 @ Ws                                            # [B, F] f32
    out = np.empty((B, ROWS, F), np.float32)
    for c in range(NCORES):
        r = res.results[c]
        gsl = slice(c * BG, (c + 1) * BG)
        out[gsl] = np.asarray(r["out"]).reshape(BG, ROWS, F).astype(np.float32)
        o4 = np.asarray(r["out4"]).astype(np.float32)             # [H, BG, F]
        agg = np.concatenate(
            [o4[h, :, h * C:(h + 1) * C] for h in range(H)], axis=1)  # [BG, F]
        out[gsl, 0, :] = skip_c[gsl] + agg
    return out.reshape(B * ROWS, F)
